# revision 14
# baseline (speedup 1.0000x reference)
"""ComplexPolarAttention Trainium2 kernel.

score_ij = sum_d mag_i,d mag_j,d cos(phase_i,d - phase_j,d)
         = a_i . a_j + b_i . b_j          with a = mag*cos(phase), b = mag*sin(phase)
out_mag   = softmax(score, axis=1) @ mag
out_phase = softmax(score, axis=1) @ phase

Strategy (8 NeuronCores, SPMD, no collectives):
  - Rows (queries) sharded: core c owns queries [c*1024, (c+1)*1024).
  - Keys replicated: every core builds the full packed ab^T = [a|b]^T
    [128=2D, N] on-chip (ACT sin with per-partition bias for cos, DVE mul),
    so the two score GEMMs fuse into ONE K=128 fp32r matmul per key block.
  - Scores are computed transposed, S^T[k_blk=128, q=512] in PSUM, exp'd on
    ACT (scores bounded by D=64 < 88 so unnormalized exp cannot overflow),
    then used as the MOVING operand of the value matmuls whose stationary
    operands are natural-layout [mag|ones] (the ones column yields the
    softmax denominator for free) and phase.
  - PSUM accumulates the numerators over all 64 key blocks; the final
    divide by the denominator happens on host during the gather.
"""

import numpy as np
from contextlib import ExitStack

import concourse.bass as bass
import concourse.tile as tile
from concourse import bacc, mybir
from concourse.bass_utils import run_bass_kernel_spmd

F32 = mybir.dt.float32
F32R = mybir.dt.float32r
HALF_PI = float(np.pi / 2.0)


def build_program(n=8192, d=64, n_cores=8, enable_asserts=False):
    """Build the SPMD Bass program. Every core runs identical IR; per-core
    behavior comes only from per-core input data (the query slices)."""
    assert d == 64
    q = n // n_cores            # queries per core
    kblocks = n // 128          # key blocks of 128
    qblk = q // 2               # half processed per matmul (fp32 moving max 512)
    assert qblk <= 512 and n % 128 == 0

    nc = bacc.Bacc(
        "TRN2",
        target_bir_lowering=False,
        debug=False,
        enable_asserts=enable_asserts,
        num_devices=n_cores,
    )

    # ---- DRAM I/O ----
    # ph2: [wrap(phase^T + pi/2) | wrap(phase^T)] stacked to 128 partitions,
    # wrapped into [-pi, pi) on host (ScalarE Sin domain); sin of the top
    # half gives cos(phase), of the bottom half sin(phase).
    magt = nc.dram_tensor("magt", [d, n], F32, kind="ExternalInput").ap()
    ph2 = nc.dram_tensor("ph2", [2 * d, n], F32, kind="ExternalInput").ap()
    magt_q = nc.dram_tensor("magt_q", [d, q], F32, kind="ExternalInput").ap()
    ph2_q = nc.dram_tensor("ph2_q", [2 * d, q], F32, kind="ExternalInput").ap()
    # [mag | ones] value matrix, pre-tiled on host to [128, kblocks*65]
    mo = nc.dram_tensor("mo", [128, kblocks * 65], F32R, kind="ExternalInput").ap()
    # phase value matrix, pre-tiled on host to [128, kblocks*64]
    pv = nc.dram_tensor("pv", [128, kblocks * d], F32R, kind="ExternalInput").ap()

    om = nc.dram_tensor("om", [65, q], F32, kind="ExternalOutput").ap()
    op = nc.dram_tensor("op", [d, q], F32, kind="ExternalOutput").ap()

    with tile.TileContext(nc) as tc, ExitStack() as ctx:
        const = ctx.enter_context(tc.tile_pool(name="const", bufs=1))
        persist = ctx.enter_context(tc.tile_pool(name="persist", bufs=1))
        bpool = ctx.enter_context(tc.tile_pool(name="build", bufs=3))
        epool = ctx.enter_context(tc.tile_pool(name="exps", bufs=4))
        opool = ctx.enter_context(tc.tile_pool(name="outs", bufs=2))
        spool = ctx.enter_context(tc.tile_pool(name="scores", bufs=2, space="PSUM"))
        apool = ctx.enter_context(tc.tile_pool(name="accum", bufs=1, space="PSUM"))

        abt = persist.tile([128, n], F32R)       # [a|b]^T for all keys
        abq = persist.tile([128, q], F32R)       # [a|b]^T for this core's queries
        mo_t = persist.tile([128, kblocks, 65], F32R)
        pv_t = persist.tile([128, kblocks, d], F32R)

        # value matrices on the gpsimd DMA queue (parallel with the sync
        # queue's ab^T input stream), chunked so early key blocks unblock
        # the first value matmuls quickly
        mo_r = mo.rearrange("p (b m) -> p b m", m=65)
        pv_r = pv.rearrange("p (b m) -> p b m", m=d)
        vchunk = max(1, kblocks // 8)
        for b0 in range(0, kblocks, vchunk):
            b1 = min(kblocks, b0 + vchunk)
            nc.gpsimd.dma_start(out=mo_t[:, b0:b1, :], in_=mo_r[:, b0:b1, :])
            nc.gpsimd.dma_start(out=pv_t[:, b0:b1, :], in_=pv_r[:, b0:b1, :])

        # ---- build ab^T in chunks: ab[0:64] = mag*cos(phase), ab[64:128] = mag*sin(phase)
        def build_ab(dst, src_m, src_p2, width, chunk):
            for c0 in range(0, width, chunk):
                sl = slice(c0, c0 + chunk)
                mg = bpool.tile([128, chunk], F32, tag="mg")
                nc.sync.dma_start(out=mg[0:64, :], in_=src_m[:, sl])
                nc.sync.dma_start(out=mg[64:128, :], in_=src_m[:, sl])
                ph = bpool.tile([128, chunk], F32, tag="ph")
                nc.sync.dma_start(out=ph[:, :], in_=src_p2[:, sl])
                tr = bpool.tile([128, chunk], F32, tag="tr")
                nc.scalar.activation(
                    tr[:, :], ph[:, :], mybir.ActivationFunctionType.Sin,
                )
                nc.vector.tensor_mul(dst[:, sl], mg[:, :], tr[:, :])

        build_ab(abq, magt_q, ph2_q, q, min(1024, q))
        build_ab(abt, magt, ph2, n, min(2048, n))

        # ---- main loop: all q (both 512-halves) in flight at once.
        # Per key block kb: one stationary load of abt_k shared by the two
        # score matmuls (q halves), ONE wide [128, q] exp (amortizes ACT's
        # ~352-cycle instruction overhead), and value matmuls whose
        # stationary mo_t/pv_t loads are likewise shared across q halves.
        # PSUM budget: scores [128,1024]x2bufs = 4 banks, psA/psB = 4 banks.
        assert q == 2 * qblk
        # one accumulator tile (= one PSUM bank / zero-region) per q half
        psA = [apool.tile([65, qblk], F32, name=f"psA{j}", tag=f"psA{j}") for j in range(2)]
        psB = [apool.tile([64, qblk], F32, name=f"psB{j}", tag=f"psB{j}") for j in range(2)]

        def value_mms(es, kb, first, last):
            for m_t, ps in ((mo_t, psA), (pv_t, psB)):
                for j in range(2):
                    nc.tensor.matmul(
                        out=ps[j][:, :],
                        lhsT=m_t[:, kb, :],
                        rhs=es[:, j * qblk:(j + 1) * qblk],
                        start=first, stop=last,
                    )

        es_prev = None
        for kb in range(kblocks):
            ss = spool.tile([128, q], F32)
            for j in range(2):
                nc.tensor.matmul(
                    out=ss[:, j * qblk:(j + 1) * qblk],
                    lhsT=abt[:, kb * 128:(kb + 1) * 128],
                    rhs=abq[:, j * qblk:(j + 1) * qblk],
                    start=True, stop=True,
                )
            es = epool.tile([128, q], F32R)
            nc.scalar.activation(
                es[:, :], ss[:, :], mybir.ActivationFunctionType.Exp,
            )
            if es_prev is not None:
                value_mms(es_prev, kb - 1, first=(kb == 1), last=False)
            es_prev = es
        value_mms(es_prev, kblocks - 1, first=False, last=True)

        for j in range(2):
            qsl = slice(j * qblk, (j + 1) * qblk)
            oA = opool.tile([65, qblk], F32, tag="oA")
            nc.vector.tensor_copy(oA[:, :], psA[j][:, :])
            nc.sync.dma_start(out=om[:, qsl], in_=oA[:, :])
            oB = opool.tile([64, qblk], F32, tag="oB")
            nc.vector.tensor_copy(oB[:, :], psB[j][:, :])
            nc.sync.dma_start(out=op[:, qsl], in_=oB[:, :])

    nc.compile()
    return nc


def make_inputs(mag, phase, n_cores=8):
    """Host-side sharding/layout prep -> per-core input maps."""
    n, d = mag.shape
    q = n // n_cores
    kblocks = n // 128
    mag = np.ascontiguousarray(mag, dtype=np.float32)
    phase = np.ascontiguousarray(phase, dtype=np.float32)
    magt = np.ascontiguousarray(mag.T)

    def wrap(x):
        # into [-pi, pi) -- ScalarE Sin domain
        return ((x + np.pi) % (2.0 * np.pi) - np.pi).astype(np.float32)

    ph2 = np.concatenate([wrap(phase.T + HALF_PI), wrap(phase.T)], axis=0)
    ph2 = np.ascontiguousarray(ph2)
    mo = np.concatenate([mag, np.ones((n, 1), np.float32)], axis=1)
    mo = np.ascontiguousarray(
        mo.reshape(kblocks, 128, 65).transpose(1, 0, 2).reshape(128, -1))
    pv = np.ascontiguousarray(
        phase.reshape(kblocks, 128, d).transpose(1, 0, 2).reshape(128, -1))
    in_maps = []
    for c in range(n_cores):
        qsl = slice(c * q, (c + 1) * q)
        in_maps.append({
            "magt": magt,
            "ph2": ph2,
            "magt_q": np.ascontiguousarray(magt[:, qsl]),
            "ph2_q": np.ascontiguousarray(ph2[:, qsl]),
            "mo": mo,
            "pv": pv,
        })
    return in_maps


def gather_outputs(results, n, d, n_cores=8):
    """Per-core [65,q]/[64,q] transposed unnormalized sums -> full outputs."""
    new_mag = np.empty((n, d), np.float32)
    new_phase = np.empty((n, d), np.float32)
    q = n // n_cores
    for c in range(n_cores):
        om = results[c]["om"]          # [65, q]
        op = results[c]["op"]          # [64, q]
        den = om[64:65, :]             # [1, q]
        qsl = slice(c * q, (c + 1) * q)
        new_mag[qsl] = (om[:64, :] / den).T
        new_phase[qsl] = (op / den).T
    return new_mag, new_phase


_PROGRAM_CACHE = {}


def _get_program(n, d, n_cores):
    key = (n, d, n_cores)
    if key not in _PROGRAM_CACHE:
        _PROGRAM_CACHE[key] = build_program(n=n, d=d, n_cores=n_cores)
    return _PROGRAM_CACHE[key]


def kernel(mag, phase):
    mag = np.asarray(mag, dtype=np.float32)
    phase = np.asarray(phase, dtype=np.float32)
    n, d = mag.shape
    n_cores = 8
    nc = _get_program(n, d, n_cores)
    in_maps = make_inputs(mag, phase, n_cores=n_cores)
    res = run_bass_kernel_spmd(nc, in_maps, list(range(n_cores)))
    return gather_outputs(res.results, n, d, n_cores=n_cores)


# revision 21
# speedup vs baseline: 1.0691x; 1.0691x over previous
"""ComplexPolarAttention Trainium2 kernel.

score_ij = sum_d mag_i,d mag_j,d cos(phase_i,d - phase_j,d)
         = a_i . a_j + b_i . b_j          with a = mag*cos(phase), b = mag*sin(phase)
out_mag   = softmax(score, axis=1) @ mag
out_phase = softmax(score, axis=1) @ phase

Strategy (8 NeuronCores, SPMD, no collectives):
  - Rows (queries) sharded: core c owns queries [c*1024, (c+1)*1024).
  - Keys replicated: every core builds the full packed ab^T = [a|b]^T
    [128=2D, N] on-chip (ACT sin with per-partition bias for cos, DVE mul),
    so the two score GEMMs fuse into ONE K=128 fp32r matmul per key block.
  - Scores are computed transposed, S^T[k_blk=128, q=512] in PSUM, exp'd on
    ACT (scores bounded by D=64 < 88 so unnormalized exp cannot overflow),
    then used as the MOVING operand of the value matmuls whose stationary
    operands are natural-layout [mag|ones] (the ones column yields the
    softmax denominator for free) and phase.
  - PSUM accumulates the numerators over all 64 key blocks; the final
    divide by the denominator happens on host during the gather.
"""

import numpy as np
from contextlib import ExitStack

import concourse.bass as bass
import concourse.tile as tile
from concourse import bacc, mybir
from concourse.bass_utils import run_bass_kernel_spmd

F32 = mybir.dt.float32
F32R = mybir.dt.float32r
HALF_PI = float(np.pi / 2.0)


def build_program(n=8192, d=64, n_cores=8, enable_asserts=False):
    """Build the SPMD Bass program. Every core runs identical IR; per-core
    behavior comes only from per-core input data (the query slices)."""
    assert d == 64
    q = n // n_cores            # queries per core
    kblocks = n // 128          # key blocks of 128
    qblk = q // 2               # half processed per matmul (fp32 moving max 512)
    assert qblk <= 512 and n % 128 == 0

    nc = bacc.Bacc(
        "TRN2",
        target_bir_lowering=False,
        debug=False,
        enable_asserts=enable_asserts,
        num_devices=n_cores,
    )

    # ---- DRAM I/O ----
    # ph2: [wrap(phase^T + pi/2) | wrap(phase^T)] stacked to 128 partitions,
    # wrapped into [-pi, pi) on host (ScalarE Sin domain); sin of the top
    # half gives cos(phase), of the bottom half sin(phase).
    magt = nc.dram_tensor("magt", [d, n], F32, kind="ExternalInput").ap()
    ph2 = nc.dram_tensor("ph2", [2 * d, n], F32, kind="ExternalInput").ap()
    magt_q = nc.dram_tensor("magt_q", [d, q], F32, kind="ExternalInput").ap()
    ph2_q = nc.dram_tensor("ph2_q", [2 * d, q], F32, kind="ExternalInput").ap()
    # [mag | ones] value matrix, pre-tiled on host to [128, kblocks*65]
    mo = nc.dram_tensor("mo", [128, kblocks * 65], F32R, kind="ExternalInput").ap()
    # phase value matrix, pre-tiled on host to [128, kblocks*64]
    pv = nc.dram_tensor("pv", [128, kblocks * d], F32R, kind="ExternalInput").ap()

    om = nc.dram_tensor("om", [65, q], F32, kind="ExternalOutput").ap()
    op = nc.dram_tensor("op", [d, q], F32, kind="ExternalOutput").ap()

    with tile.TileContext(nc) as tc, ExitStack() as ctx:
        const = ctx.enter_context(tc.tile_pool(name="const", bufs=1))
        persist = ctx.enter_context(tc.tile_pool(name="persist", bufs=1))
        bpool = ctx.enter_context(tc.tile_pool(name="build", bufs=3))
        epool = ctx.enter_context(tc.tile_pool(name="exps", bufs=6))
        opool = ctx.enter_context(tc.tile_pool(name="outs", bufs=2))
        spool = ctx.enter_context(tc.tile_pool(name="scores", bufs=2, space="PSUM"))
        apool = ctx.enter_context(tc.tile_pool(name="accum", bufs=1, space="PSUM"))

        abt = persist.tile([128, n], F32R)       # [a|b]^T for all keys
        abq = persist.tile([128, q], F32R)       # [a|b]^T for this core's queries
        mo_t = persist.tile([128, kblocks, 65], F32R)
        pv_t = persist.tile([128, kblocks, d], F32R)

        # ---- build ab^T in chunks: ab[0:64] = mag*cos(phase), ab[64:128] = mag*sin(phase)
        # ph2 loads ride the sync queue (they gate the ACT sin stream);
        # mag loads ride the gpsimd SWDGE queue so they can't delay the phases.
        sin_insts = []

        def build_ab(dst, src_m, src_p2, width, chunk):
            for c0 in range(0, width, chunk):
                sl = slice(c0, c0 + chunk)
                ph = bpool.tile([128, chunk], F32, tag="ph")
                nc.sync.dma_start(out=ph[:, :], in_=src_p2[:, sl])
                mg = bpool.tile([128, chunk], F32, tag="mg")
                nc.gpsimd.dma_start(out=mg[0:64, :], in_=src_m[:, sl])
                nc.gpsimd.dma_start(out=mg[64:128, :], in_=src_m[:, sl])
                tr = bpool.tile([128, chunk], F32, tag="tr")
                sin_insts.append(nc.scalar.activation(
                    tr[:, :], ph[:, :], mybir.ActivationFunctionType.Sin,
                ))
                nc.vector.tensor_mul(dst[:, sl], mg[:, :], tr[:, :])

        build_ab(abq, magt_q, ph2_q, q, min(1024, q))
        build_ab(abt, magt, ph2, n, min(2048, n))

        bchunk = min(2048, n)
        nbch = n // bchunk
        abt_tails = abt[:, bchunk - 1::bchunk]    # [128, n/bchunk] strided

        # value matrices on the gpsimd DMA queue behind the mag loads,
        # chunked so early key blocks unblock the first value matmuls
        mo_r = mo.rearrange("p (b m) -> p b m", m=65)
        pv_r = pv.rearrange("p (b m) -> p b m", m=d)
        vchunk = max(1, kblocks // 8)
        for b0 in range(0, kblocks, vchunk):
            b1 = min(kblocks, b0 + vchunk)
            nc.gpsimd.dma_start(out=mo_t[:, b0:b1, :], in_=mo_r[:, b0:b1, :])
            nc.gpsimd.dma_start(out=pv_t[:, b0:b1, :], in_=pv_r[:, b0:b1, :])

        # ---- main loop: all q (both 512-halves) in flight at once.
        # Per key block kb: one stationary load of abt_k shared by the two
        # score matmuls (q halves), ONE wide [128, q] exp (amortizes ACT's
        # ~352-cycle instruction overhead), and value matmuls whose
        # stationary mo_t/pv_t loads are likewise shared across q halves.
        # PSUM budget: scores [128,1024]x2bufs = 4 banks, psA/psB = 4 banks.
        assert q == 2 * qblk
        # one accumulator tile (= one PSUM bank / zero-region) per q half
        psA = [apool.tile([65, qblk], F32, name=f"psA{j}", tag=f"psA{j}") for j in range(2)]
        psB = [apool.tile([64, qblk], F32, name=f"psB{j}", tag=f"psB{j}") for j in range(2)]

        def value_mms(es, kb, first, last):
            for m_t, ps in ((mo_t, psA), (pv_t, psB)):
                for j in range(2):
                    nc.tensor.matmul(
                        out=ps[j][:, :],
                        lhsT=m_t[:, kb, :],
                        rhs=es[:, j * qblk:(j + 1) * qblk],
                        start=first, stop=last,
                    )

        # zero bias tile whose producer reads the tail column of every abt
        # chunk -> transitively depends on every Sin (ACT table grouping)
        zb = const.tile([128, 1], F32)
        ztmp = const.tile([128, nbch], F32)
        nc.vector.tensor_scalar_mul(ztmp[:, :], abt_tails[:, :], 0.0)
        nc.vector.tensor_reduce(
            out=zb[:, :], in_=ztmp[:, :], axis=mybir.AxisListType.X,
            op=mybir.AluOpType.add)

        es_hist = []
        for kb in range(kblocks):
            ss = spool.tile([128, q], F32)
            for j in range(2):
                nc.tensor.matmul(
                    out=ss[:, j * qblk:(j + 1) * qblk],
                    lhsT=abt[:, kb * 128:(kb + 1) * 128],
                    rhs=abq[:, j * qblk:(j + 1) * qblk],
                    start=True, stop=True,
                )
            es = epool.tile([128, q], F32R)
            # zb (zeros, produced downstream of every sin) as the bias AP
            # keeps all Sins scheduled before the first Exp on ACT -- Sin and
            # Exp live in different ACT table sets, so interleaving would pay
            # a ~2.7us table reload per switch
            nc.scalar.activation(
                es[:, :], ss[:, :], mybir.ActivationFunctionType.Exp,
                bias=zb[:, :],
            )
            if len(es_hist) >= 2:
                value_mms(es_hist[-2], kb - 2, first=(kb == 2), last=False)
            es_hist.append(es)
        value_mms(es_hist[-2], kblocks - 2, first=False, last=False)
        value_mms(es_hist[-1], kblocks - 1, first=False, last=True)

        for j in range(2):
            qsl = slice(j * qblk, (j + 1) * qblk)
            oA = opool.tile([65, qblk], F32, tag="oA")
            nc.vector.tensor_copy(oA[:, :], psA[j][:, :])
            nc.sync.dma_start(out=om[:, qsl], in_=oA[:, :])
            oB = opool.tile([64, qblk], F32, tag="oB")
            nc.vector.tensor_copy(oB[:, :], psB[j][:, :])
            nc.sync.dma_start(out=op[:, qsl], in_=oB[:, :])

    nc.compile()
    return nc


def make_inputs(mag, phase, n_cores=8):
    """Host-side sharding/layout prep -> per-core input maps."""
    n, d = mag.shape
    q = n // n_cores
    kblocks = n // 128
    mag = np.ascontiguousarray(mag, dtype=np.float32)
    phase = np.ascontiguousarray(phase, dtype=np.float32)
    magt = np.ascontiguousarray(mag.T)

    def wrap(x):
        # into [-pi, pi) -- ScalarE Sin domain
        return ((x + np.pi) % (2.0 * np.pi) - np.pi).astype(np.float32)

    ph2 = np.concatenate([wrap(phase.T + HALF_PI), wrap(phase.T)], axis=0)
    ph2 = np.ascontiguousarray(ph2)
    mo = np.concatenate([mag, np.ones((n, 1), np.float32)], axis=1)
    mo = np.ascontiguousarray(
        mo.reshape(kblocks, 128, 65).transpose(1, 0, 2).reshape(128, -1))
    pv = np.ascontiguousarray(
        phase.reshape(kblocks, 128, d).transpose(1, 0, 2).reshape(128, -1))
    in_maps = []
    for c in range(n_cores):
        qsl = slice(c * q, (c + 1) * q)
        in_maps.append({
            "magt": magt,
            "ph2": ph2,
            "magt_q": np.ascontiguousarray(magt[:, qsl]),
            "ph2_q": np.ascontiguousarray(ph2[:, qsl]),
            "mo": mo,
            "pv": pv,
        })
    return in_maps


def gather_outputs(results, n, d, n_cores=8):
    """Per-core [65,q]/[64,q] transposed unnormalized sums -> full outputs."""
    new_mag = np.empty((n, d), np.float32)
    new_phase = np.empty((n, d), np.float32)
    q = n // n_cores
    for c in range(n_cores):
        om = results[c]["om"]          # [65, q]
        op = results[c]["op"]          # [64, q]
        den = om[64:65, :]             # [1, q]
        qsl = slice(c * q, (c + 1) * q)
        new_mag[qsl] = (om[:64, :] / den).T
        new_phase[qsl] = (op / den).T
    return new_mag, new_phase


_PROGRAM_CACHE = {}


def _get_program(n, d, n_cores):
    key = (n, d, n_cores)
    if key not in _PROGRAM_CACHE:
        _PROGRAM_CACHE[key] = build_program(n=n, d=d, n_cores=n_cores)
    return _PROGRAM_CACHE[key]


def kernel(mag, phase):
    mag = np.asarray(mag, dtype=np.float32)
    phase = np.asarray(phase, dtype=np.float32)
    n, d = mag.shape
    n_cores = 8
    nc = _get_program(n, d, n_cores)
    in_maps = make_inputs(mag, phase, n_cores=n_cores)
    res = run_bass_kernel_spmd(nc, in_maps, list(range(n_cores)))
    return gather_outputs(res.results, n, d, n_cores=n_cores)


# revision 23
# speedup vs baseline: 1.1850x; 1.1084x over previous
"""ComplexPolarAttention Trainium2 kernel.

score_ij = sum_d mag_i,d mag_j,d cos(phase_i,d - phase_j,d)
         = a_i . a_j + b_i . b_j          with a = mag*cos(phase), b = mag*sin(phase)
out_mag   = softmax(score, axis=1) @ mag
out_phase = softmax(score, axis=1) @ phase

Strategy (8 NeuronCores, SPMD, no collectives):
  - Rows (queries) sharded; keys replicated. Per-core inputs are ROTATED
    along the key axis so that core c's queries are always columns 0..q of
    its own key panel (softmax over keys is permutation invariant), so the
    query operand is just a slice of the key panel.
  - Every core builds the packed ab^T = [a|b]^T [128=2D, N] on-chip (one
    ACT Sin over host-prewrapped fp16 phase args, one DVE mul with fp16
    mags), fusing the two score GEMMs into ONE K=128 fp32r matmul per key
    block of 128.
  - Scores are computed transposed, S^T[k_blk=128, q] in PSUM (one wide
    [128, 1024] exp per key block amortizes ACT's ~352-cycle instruction
    overhead; scores are bounded by D=64 < 88 so unnormalized exp cannot
    overflow), then used as the MOVING operand of the value matmuls whose
    stationary operands are natural-layout [mag|ones] (the ones column
    yields the softmax denominator for free) and phase.
  - PSUM accumulates the numerators over all 64 key blocks; the final
    divide by the denominator happens on host during the gather.
"""

import numpy as np
from contextlib import ExitStack

import concourse.bass as bass
import concourse.tile as tile
from concourse import bacc, mybir
from concourse.bass_utils import run_bass_kernel_spmd

F32 = mybir.dt.float32
F32R = mybir.dt.float32r
F16 = mybir.dt.float16
HALF_PI = float(np.pi / 2.0)


def build_program(n=8192, d=64, n_cores=8, enable_asserts=False):
    """Build the SPMD Bass program. Every core runs identical IR; per-core
    behavior comes only from per-core (rotated) input data."""
    assert d == 64
    q = n // n_cores            # queries per core
    kblocks = n // 128          # key blocks of 128
    qblk = q // 2               # half processed per matmul (fp32 moving max 512)
    assert qblk <= 512 and n % 128 == 0

    nc = bacc.Bacc(
        "TRN2",
        target_bir_lowering=False,
        debug=False,
        enable_asserts=enable_asserts,
        num_devices=n_cores,
    )

    # ---- DRAM I/O (all per-core arrays rotated so queries = keys[0:q]) ----
    # ph2: [wrap(phase^T + pi/2) | wrap(phase^T)] stacked to 128 partitions,
    # wrapped into [-pi, pi) on host (ScalarE Sin domain), fp16.
    # mg2: [mag^T | mag^T] stacked, fp16.
    ph2 = nc.dram_tensor("ph2", [2 * d, n], F16, kind="ExternalInput").ap()
    mg2 = nc.dram_tensor("mg2", [2 * d, n], F16, kind="ExternalInput").ap()
    # [mag | ones] value matrix, pre-tiled on host to [128, kblocks*65]
    mo = nc.dram_tensor("mo", [128, kblocks * 65], F32R, kind="ExternalInput").ap()
    # phase value matrix, pre-tiled on host to [128, kblocks*64]
    pv = nc.dram_tensor("pv", [128, kblocks * d], F32R, kind="ExternalInput").ap()

    om = nc.dram_tensor("om", [65, q], F32, kind="ExternalOutput").ap()
    op = nc.dram_tensor("op", [d, q], F32, kind="ExternalOutput").ap()

    with tile.TileContext(nc) as tc, ExitStack() as ctx:
        const = ctx.enter_context(tc.tile_pool(name="const", bufs=1))
        persist = ctx.enter_context(tc.tile_pool(name="persist", bufs=1))
        bpool = ctx.enter_context(tc.tile_pool(name="build", bufs=3))
        epool = ctx.enter_context(tc.tile_pool(name="exps", bufs=6))
        spool = ctx.enter_context(tc.tile_pool(name="scores", bufs=2, space="PSUM"))
        apool = ctx.enter_context(tc.tile_pool(name="accum", bufs=1, space="PSUM"))

        abt = persist.tile([128, n], F32R)       # [a|b]^T for all keys
        mo_t = persist.tile([128, kblocks, 65], F32R)
        pv_t = persist.tile([128, kblocks, d], F32R)

        # ---- build ab^T: ab[0:64] = mag*cos(phase), ab[64:128] = mag*sin(phase)
        # phase args ride the sync HWDGE queue, mags the gpsimd SWDGE queue.
        bchunk = min(2048, n)
        nbch = n // bchunk
        for c0 in range(0, n, bchunk):
            sl = slice(c0, c0 + bchunk)
            ph = bpool.tile([128, bchunk], F16, tag="ph")
            nc.sync.dma_start(out=ph[:, :], in_=ph2[:, sl])
            mg = bpool.tile([128, bchunk], F16, tag="mg")
            nc.gpsimd.dma_start(out=mg[:, :], in_=mg2[:, sl])
            tr = bpool.tile([128, bchunk], F32, tag="tr")
            nc.scalar.activation(
                tr[:, :], ph[:, :], mybir.ActivationFunctionType.Sin,
            )
            nc.vector.tensor_mul(abt[:, sl], mg[:, :], tr[:, :])

        abq = abt[:, 0:q]          # queries are the first q key columns

        # value matrices: mo behind the phases on sync, pv behind the mags
        # on gpsimd, chunked so early key blocks unblock the first value
        # matmuls quickly
        mo_r = mo.rearrange("p (b m) -> p b m", m=65)
        pv_r = pv.rearrange("p (b m) -> p b m", m=d)
        vchunk = max(1, kblocks // 8)
        for b0 in range(0, kblocks, vchunk):
            b1 = min(kblocks, b0 + vchunk)
            nc.sync.dma_start(out=mo_t[:, b0:b1, :], in_=mo_r[:, b0:b1, :])
            nc.gpsimd.dma_start(out=pv_t[:, b0:b1, :], in_=pv_r[:, b0:b1, :])

        # ---- main loop: all q (both 512-halves) in flight at once.
        # Per key block kb: one stationary load of abt_k shared by the two
        # score matmuls (q halves), ONE wide [128, q] exp, value matmuls two
        # key blocks behind (their es-ready semaphores are then already
        # satisfied when the weight loads issue).
        # PSUM budget: scores [128,1024]x2bufs = 4 banks, psA/psB = 4 banks.
        assert q == 2 * qblk
        # one accumulator tile (= one PSUM bank / zero-region) per q half
        psA = [apool.tile([65, qblk], F32, name=f"psA{j}", tag=f"psA{j}")
               for j in range(2)]
        psB = [apool.tile([64, qblk], F32, name=f"psB{j}", tag=f"psB{j}")
               for j in range(2)]

        def value_mms(es, kb, first, last):
            for m_t, ps in ((mo_t, psA), (pv_t, psB)):
                for j in range(2):
                    nc.tensor.matmul(
                        out=ps[j][:, :],
                        lhsT=m_t[:, kb, :],
                        rhs=es[:, j * qblk:(j + 1) * qblk],
                        start=first, stop=last,
                    )

        # zero bias tile whose producer reads the tail column of every abt
        # chunk -> transitively depends on every Sin. This keeps all Sins
        # scheduled before the first Exp on ACT: Sin and Exp live in
        # different ACT table sets, so interleaving costs a ~2.7us table
        # reload per switch.
        abt_tails = abt[:, bchunk - 1::bchunk]    # [128, nbch] strided
        zb = const.tile([128, 1], F32)
        ztmp = const.tile([128, nbch], F32)
        nc.vector.tensor_scalar_mul(ztmp[:, :], abt_tails[:, :], 0.0)
        nc.vector.tensor_reduce(
            out=zb[:, :], in_=ztmp[:, :], axis=mybir.AxisListType.X,
            op=mybir.AluOpType.add)

        es_hist = []
        for kb in range(kblocks):
            ss = spool.tile([128, q], F32)
            for j in range(2):
                nc.tensor.matmul(
                    out=ss[:, j * qblk:(j + 1) * qblk],
                    lhsT=abt[:, kb * 128:(kb + 1) * 128],
                    rhs=abq[:, j * qblk:(j + 1) * qblk],
                    start=True, stop=True,
                )
            es = epool.tile([128, q], F32R)
            nc.scalar.activation(
                es[:, :], ss[:, :], mybir.ActivationFunctionType.Exp,
                bias=zb[:, :],
            )
            if len(es_hist) >= 2:
                value_mms(es_hist[-2], kb - 2, first=(kb == 2), last=False)
            es_hist.append(es)
        value_mms(es_hist[-2], kblocks - 2, first=False, last=False)
        value_mms(es_hist[-1], kblocks - 1, first=False, last=True)

        # outputs: PSUM -> SBUF (DVE) -> DRAM
        opool = ctx.enter_context(tc.tile_pool(name="outs", bufs=2))
        for j in range(2):
            qsl = slice(j * qblk, (j + 1) * qblk)
            oA = opool.tile([65, qblk], F32, tag="oA")
            nc.vector.tensor_copy(oA[:, :], psA[j][:, :])
            nc.sync.dma_start(out=om[:, qsl], in_=oA[:, :])
            oB = opool.tile([64, qblk], F32, tag="oB")
            nc.vector.tensor_copy(oB[:, :], psB[j][:, :])
            nc.sync.dma_start(out=op[:, qsl], in_=oB[:, :])

    nc.compile()
    return nc


def make_inputs(mag, phase, n_cores=8):
    """Host-side sharding/layout prep -> per-core (key-rotated) input maps."""
    n, d = mag.shape
    q = n // n_cores
    kblocks = n // 128
    mag = np.ascontiguousarray(mag, dtype=np.float32)
    phase = np.ascontiguousarray(phase, dtype=np.float32)

    def wrap(x):
        # into [-pi, pi) -- ScalarE Sin domain
        return ((x + np.pi) % (2.0 * np.pi) - np.pi).astype(np.float32)

    ph2_g = np.concatenate(
        [wrap(phase.T + HALF_PI), wrap(phase.T)], axis=0).astype(np.float16)
    mg2_g = np.concatenate([mag.T, mag.T], axis=0).astype(np.float16)
    mo_nat = np.concatenate([mag, np.ones((n, 1), np.float32)], axis=1)

    def tile_nat(x):  # [n, m] -> [128, kblocks*m]
        m = x.shape[1]
        return np.ascontiguousarray(
            x.reshape(kblocks, 128, m).transpose(1, 0, 2).reshape(128, -1))

    in_maps = []
    for c in range(n_cores):
        r = c * q
        in_maps.append({
            "ph2": np.ascontiguousarray(np.roll(ph2_g, -r, axis=1)),
            "mg2": np.ascontiguousarray(np.roll(mg2_g, -r, axis=1)),
            "mo": tile_nat(np.roll(mo_nat, -r, axis=0)),
            "pv": tile_nat(np.roll(phase, -r, axis=0)),
        })
    return in_maps


def gather_outputs(results, n, d, n_cores=8):
    """Per-core [65,q]/[64,q] transposed unnormalized sums -> full outputs."""
    new_mag = np.empty((n, d), np.float32)
    new_phase = np.empty((n, d), np.float32)
    q = n // n_cores
    for c in range(n_cores):
        om = results[c]["om"]          # [65, q]
        op = results[c]["op"]          # [64, q]
        den = om[64:65, :]             # [1, q]
        qsl = slice(c * q, (c + 1) * q)
        new_mag[qsl] = (om[:64, :] / den).T
        new_phase[qsl] = (op / den).T
    return new_mag, new_phase


_PROGRAM_CACHE = {}


def _get_program(n, d, n_cores):
    key = (n, d, n_cores)
    if key not in _PROGRAM_CACHE:
        _PROGRAM_CACHE[key] = build_program(n=n, d=d, n_cores=n_cores)
    return _PROGRAM_CACHE[key]


def kernel(mag, phase):
    mag = np.asarray(mag, dtype=np.float32)
    phase = np.asarray(phase, dtype=np.float32)
    n, d = mag.shape
    n_cores = 8
    nc = _get_program(n, d, n_cores)
    in_maps = make_inputs(mag, phase, n_cores=n_cores)
    res = run_bass_kernel_spmd(nc, in_maps, list(range(n_cores)))
    return gather_outputs(res.results, n, d, n_cores=n_cores)


# revision 24
# speedup vs baseline: 1.2081x; 1.0195x over previous
"""ComplexPolarAttention Trainium2 kernel.

score_ij = sum_d mag_i,d mag_j,d cos(phase_i,d - phase_j,d)
         = a_i . a_j + b_i . b_j          with a = mag*cos(phase), b = mag*sin(phase)
out_mag   = softmax(score, axis=1) @ mag
out_phase = softmax(score, axis=1) @ phase

Strategy (8 NeuronCores, SPMD, no collectives):
  - Rows (queries) sharded; keys replicated. Per-core inputs are ROTATED
    along the key axis so that core c's queries are always columns 0..q of
    its own key panel (softmax over keys is permutation invariant), so the
    query operand is just a slice of the key panel.
  - Every core builds the packed ab^T = [a|b]^T [128=2D, N] on-chip (one
    ACT Sin over host-prewrapped fp16 phase args, one DVE mul with fp16
    mags), fusing the two score GEMMs into ONE K=128 fp32r matmul per key
    block of 128.
  - Scores are computed transposed, S^T[k_blk=128, q] in PSUM (one wide
    [128, 1024] exp per key block amortizes ACT's ~352-cycle instruction
    overhead; scores are bounded by D=64 < 88 so unnormalized exp cannot
    overflow), then used as the MOVING operand of the value matmuls whose
    stationary operands are natural-layout [mag|ones] (the ones column
    yields the softmax denominator for free) and phase.
  - PSUM accumulates the numerators over all 64 key blocks; the final
    divide by the denominator happens on host during the gather.
"""

import numpy as np
from contextlib import ExitStack

import concourse.bass as bass
import concourse.tile as tile
from concourse import bacc, mybir
from concourse.bass_utils import run_bass_kernel_spmd

F32 = mybir.dt.float32
F32R = mybir.dt.float32r
F16 = mybir.dt.float16
HALF_PI = float(np.pi / 2.0)


def build_program(n=8192, d=64, n_cores=8, enable_asserts=False):
    """Build the SPMD Bass program. Every core runs identical IR; per-core
    behavior comes only from per-core (rotated) input data."""
    assert d == 64
    q = n // n_cores            # queries per core
    kblocks = n // 128          # key blocks of 128
    qblk = q // 2               # half processed per matmul (fp32 moving max 512)
    assert qblk <= 512 and n % 128 == 0

    nc = bacc.Bacc(
        "TRN2",
        target_bir_lowering=False,
        debug=False,
        enable_asserts=enable_asserts,
        num_devices=n_cores,
    )

    # ---- DRAM I/O (all per-core arrays rotated so queries = keys[0:q]) ----
    # ph2: [wrap(phase^T + pi/2) | wrap(phase^T)] stacked to 128 partitions,
    # wrapped into [-pi, pi) on host (ScalarE Sin domain), fp16.
    # mg2: [mag^T | mag^T] stacked, fp16.
    bchunk = min(2048, n)
    nbch = n // bchunk
    vchunk = max(1, kblocks // 8)
    nvch = kblocks // vchunk
    # all inputs chunk-major so every dma_start reads one fully contiguous
    # DRAM block (strided per-partition slices are descriptor-dominated)
    ph2 = nc.dram_tensor("ph2", [nbch, 2 * d, bchunk], F16,
                         kind="ExternalInput").ap()
    mg2 = nc.dram_tensor("mg2", [nbch, 2 * d, bchunk], F16,
                         kind="ExternalInput").ap()
    # [mag | ones] value matrix, chunk-major [nvch, 128, vchunk*65]
    mo = nc.dram_tensor("mo", [nvch, 128, vchunk * 65], F32R,
                        kind="ExternalInput").ap()
    # phase value matrix, chunk-major [nvch, 128, vchunk*64]
    pv = nc.dram_tensor("pv", [nvch, 128, vchunk * d], F32R,
                        kind="ExternalInput").ap()

    om = nc.dram_tensor("om", [65, q], F32, kind="ExternalOutput").ap()
    op = nc.dram_tensor("op", [d, q], F32, kind="ExternalOutput").ap()

    with tile.TileContext(nc) as tc, ExitStack() as ctx:
        const = ctx.enter_context(tc.tile_pool(name="const", bufs=1))
        persist = ctx.enter_context(tc.tile_pool(name="persist", bufs=1))
        bpool = ctx.enter_context(tc.tile_pool(name="build", bufs=3))
        epool = ctx.enter_context(tc.tile_pool(name="exps", bufs=6))
        spool = ctx.enter_context(tc.tile_pool(name="scores", bufs=2, space="PSUM"))
        apool = ctx.enter_context(tc.tile_pool(name="accum", bufs=1, space="PSUM"))

        abt = persist.tile([128, n], F32R)       # [a|b]^T for all keys
        mo_t = persist.tile([128, kblocks, 65], F32R)
        pv_t = persist.tile([128, kblocks, d], F32R)

        # ---- build ab^T: ab[0:64] = mag*cos(phase), ab[64:128] = mag*sin(phase)
        # phase args ride the sync HWDGE queue, mags the gpsimd SWDGE queue.
        for ci in range(nbch):
            c0 = ci * bchunk
            sl = slice(c0, c0 + bchunk)
            ph = bpool.tile([128, bchunk], F16, tag="ph")
            nc.sync.dma_start(out=ph[:, :], in_=ph2[ci, :, :])
            mg = bpool.tile([128, bchunk], F16, tag="mg")
            nc.gpsimd.dma_start(out=mg[:, :], in_=mg2[ci, :, :])
            tr = bpool.tile([128, bchunk], F32, tag="tr")
            nc.scalar.activation(
                tr[:, :], ph[:, :], mybir.ActivationFunctionType.Sin,
            )
            nc.vector.tensor_mul(abt[:, sl], mg[:, :], tr[:, :])

        abq = abt[:, 0:q]          # queries are the first q key columns

        # value matrices: mo behind the phases on sync, pv behind the mags
        # on gpsimd, chunked so early key blocks unblock the first value
        # matmuls quickly
        for vi in range(nvch):
            b0 = vi * vchunk
            b1 = b0 + vchunk
            nc.sync.dma_start(out=mo_t[:, b0:b1, :], in_=mo[vi, :, :])
            nc.gpsimd.dma_start(out=pv_t[:, b0:b1, :], in_=pv[vi, :, :])

        # ---- main loop: all q (both 512-halves) in flight at once.
        # Per key block kb: one stationary load of abt_k shared by the two
        # score matmuls (q halves), ONE wide [128, q] exp, value matmuls two
        # key blocks behind (their es-ready semaphores are then already
        # satisfied when the weight loads issue).
        # PSUM budget: scores [128,1024]x2bufs = 4 banks, psA/psB = 4 banks.
        assert q == 2 * qblk
        # one accumulator tile (= one PSUM bank / zero-region) per q half
        psA = [apool.tile([65, qblk], F32, name=f"psA{j}", tag=f"psA{j}")
               for j in range(2)]
        psB = [apool.tile([64, qblk], F32, name=f"psB{j}", tag=f"psB{j}")
               for j in range(2)]

        def value_mms(es, kb, first, last):
            for m_t, ps in ((mo_t, psA), (pv_t, psB)):
                for j in range(2):
                    nc.tensor.matmul(
                        out=ps[j][:, :],
                        lhsT=m_t[:, kb, :],
                        rhs=es[:, j * qblk:(j + 1) * qblk],
                        start=first, stop=last,
                    )

        # zero bias tile whose producer reads the tail column of every abt
        # chunk -> transitively depends on every Sin. This keeps all Sins
        # scheduled before the first Exp on ACT: Sin and Exp live in
        # different ACT table sets, so interleaving costs a ~2.7us table
        # reload per switch.
        abt_tails = abt[:, bchunk - 1::bchunk]    # [128, nbch] strided
        zb = const.tile([128, 1], F32)
        ztmp = const.tile([128, nbch], F32)
        nc.vector.tensor_scalar_mul(ztmp[:, :], abt_tails[:, :], 0.0)
        nc.vector.tensor_reduce(
            out=zb[:, :], in_=ztmp[:, :], axis=mybir.AxisListType.X,
            op=mybir.AluOpType.add)

        es_hist = []
        for kb in range(kblocks):
            ss = spool.tile([128, q], F32)
            for j in range(2):
                nc.tensor.matmul(
                    out=ss[:, j * qblk:(j + 1) * qblk],
                    lhsT=abt[:, kb * 128:(kb + 1) * 128],
                    rhs=abq[:, j * qblk:(j + 1) * qblk],
                    start=True, stop=True,
                )
            es = epool.tile([128, q], F32R)
            nc.scalar.activation(
                es[:, :], ss[:, :], mybir.ActivationFunctionType.Exp,
                bias=zb[:, :],
            )
            if len(es_hist) >= 2:
                value_mms(es_hist[-2], kb - 2, first=(kb == 2), last=False)
            es_hist.append(es)
        value_mms(es_hist[-2], kblocks - 2, first=False, last=False)
        value_mms(es_hist[-1], kblocks - 1, first=False, last=True)

        # outputs: PSUM -> SBUF (DVE) -> DRAM
        opool = ctx.enter_context(tc.tile_pool(name="outs", bufs=2))
        for j in range(2):
            qsl = slice(j * qblk, (j + 1) * qblk)
            oA = opool.tile([65, qblk], F32, tag="oA")
            nc.vector.tensor_copy(oA[:, :], psA[j][:, :])
            nc.sync.dma_start(out=om[:, qsl], in_=oA[:, :])
            oB = opool.tile([64, qblk], F32, tag="oB")
            nc.vector.tensor_copy(oB[:, :], psB[j][:, :])
            nc.sync.dma_start(out=op[:, qsl], in_=oB[:, :])

    nc.compile()
    return nc


def make_inputs(mag, phase, n_cores=8):
    """Host-side sharding/layout prep -> per-core (key-rotated) input maps."""
    n, d = mag.shape
    q = n // n_cores
    kblocks = n // 128
    mag = np.ascontiguousarray(mag, dtype=np.float32)
    phase = np.ascontiguousarray(phase, dtype=np.float32)

    def wrap(x):
        # into [-pi, pi) -- ScalarE Sin domain
        return ((x + np.pi) % (2.0 * np.pi) - np.pi).astype(np.float32)

    ph2_g = np.concatenate(
        [wrap(phase.T + HALF_PI), wrap(phase.T)], axis=0).astype(np.float16)
    mg2_g = np.concatenate([mag.T, mag.T], axis=0).astype(np.float16)
    mo_nat = np.concatenate([mag, np.ones((n, 1), np.float32)], axis=1)

    bchunk = min(2048, n)
    nbch = n // bchunk
    vchunk = max(1, kblocks // 8)
    nvch = kblocks // vchunk

    def chunk_tr(x):  # [128, n] -> [nbch, 128, bchunk] chunk-major
        return np.ascontiguousarray(
            x.reshape(2 * d, nbch, bchunk).transpose(1, 0, 2))

    def tile_nat(x):  # [n, m] -> [nvch, 128, vchunk*m] chunk-major
        m = x.shape[1]
        y = x.reshape(nvch, vchunk, 128, m).transpose(0, 2, 1, 3)
        return np.ascontiguousarray(y.reshape(nvch, 128, vchunk * m))

    in_maps = []
    for c in range(n_cores):
        r = c * q
        in_maps.append({
            "ph2": chunk_tr(np.roll(ph2_g, -r, axis=1)),
            "mg2": chunk_tr(np.roll(mg2_g, -r, axis=1)),
            "mo": tile_nat(np.roll(mo_nat, -r, axis=0)),
            "pv": tile_nat(np.roll(phase, -r, axis=0)),
        })
    return in_maps


def gather_outputs(results, n, d, n_cores=8):
    """Per-core [65,q]/[64,q] transposed unnormalized sums -> full outputs."""
    new_mag = np.empty((n, d), np.float32)
    new_phase = np.empty((n, d), np.float32)
    q = n // n_cores
    for c in range(n_cores):
        om = results[c]["om"]          # [65, q]
        op = results[c]["op"]          # [64, q]
        den = om[64:65, :]             # [1, q]
        qsl = slice(c * q, (c + 1) * q)
        new_mag[qsl] = (om[:64, :] / den).T
        new_phase[qsl] = (op / den).T
    return new_mag, new_phase


_PROGRAM_CACHE = {}


def _get_program(n, d, n_cores):
    key = (n, d, n_cores)
    if key not in _PROGRAM_CACHE:
        _PROGRAM_CACHE[key] = build_program(n=n, d=d, n_cores=n_cores)
    return _PROGRAM_CACHE[key]


def kernel(mag, phase):
    mag = np.asarray(mag, dtype=np.float32)
    phase = np.asarray(phase, dtype=np.float32)
    n, d = mag.shape
    n_cores = 8
    nc = _get_program(n, d, n_cores)
    in_maps = make_inputs(mag, phase, n_cores=n_cores)
    res = run_bass_kernel_spmd(nc, in_maps, list(range(n_cores)))
    return gather_outputs(res.results, n, d, n_cores=n_cores)


# revision 25
# speedup vs baseline: 1.2590x; 1.0421x over previous
"""ComplexPolarAttention Trainium2 kernel.

score_ij = sum_d mag_i,d mag_j,d cos(phase_i,d - phase_j,d)
         = a_i . a_j + b_i . b_j          with a = mag*cos(phase), b = mag*sin(phase)
out_mag   = softmax(score, axis=1) @ mag
out_phase = softmax(score, axis=1) @ phase

Strategy (8 NeuronCores, SPMD, no collectives):
  - Rows (queries) sharded; keys replicated. Per-core inputs are ROTATED
    along the key axis so that core c's queries are always columns 0..q of
    its own key panel (softmax over keys is permutation invariant), so the
    query operand is just a slice of the key panel.
  - Every core builds the packed ab^T = [a|b]^T [128=2D, N] on-chip (one
    ACT Sin over host-prewrapped fp16 phase args, one DVE mul with fp16
    mags), fusing the two score GEMMs into ONE K=128 fp32r matmul per key
    block of 128.
  - Scores are computed transposed, S^T[k_blk=128, q] in PSUM (one wide
    [128, 1024] exp per key block amortizes ACT's ~352-cycle instruction
    overhead; scores are bounded by D=64 < 88 so unnormalized exp cannot
    overflow), then used as the MOVING operand of the value matmuls whose
    stationary operands are natural-layout [mag|ones] (the ones column
    yields the softmax denominator for free) and phase.
  - PSUM accumulates the numerators over all 64 key blocks; the final
    divide by the denominator happens on host during the gather.
"""

import numpy as np
from contextlib import ExitStack

import concourse.bass as bass
import concourse.tile as tile
from concourse import bacc, mybir
from concourse.bass_utils import run_bass_kernel_spmd

F32 = mybir.dt.float32
F32R = mybir.dt.float32r
F16 = mybir.dt.float16
HALF_PI = float(np.pi / 2.0)


def build_program(n=8192, d=64, n_cores=8, enable_asserts=False):
    """Build the SPMD Bass program. Every core runs identical IR; per-core
    behavior comes only from per-core (rotated) input data."""
    assert d == 64
    q = n // n_cores            # queries per core
    kblocks = n // 128          # key blocks of 128
    qblk = q // 2               # half processed per matmul (fp32 moving max 512)
    assert qblk <= 512 and n % 128 == 0

    nc = bacc.Bacc(
        "TRN2",
        target_bir_lowering=False,
        debug=False,
        enable_asserts=enable_asserts,
        num_devices=n_cores,
    )

    # ---- DRAM I/O (all per-core arrays rotated so queries = keys[0:q]) ----
    # ph2: [wrap(phase^T + pi/2) | wrap(phase^T)] stacked to 128 partitions,
    # wrapped into [-pi, pi) on host (ScalarE Sin domain), fp16.
    # mg2: [mag^T | mag^T] stacked, fp16.
    bchunk = min(2048, n)
    nbch = n // bchunk
    vchunk = max(1, kblocks // 8)
    nvch = kblocks // vchunk
    # all inputs chunk-major so every dma_start reads one fully contiguous
    # DRAM block (strided per-partition slices are descriptor-dominated)
    ph2 = nc.dram_tensor("ph2", [nbch, 2 * d, bchunk], F16,
                         kind="ExternalInput").ap()
    mg2 = nc.dram_tensor("mg2", [nbch, 2 * d, bchunk], F16,
                         kind="ExternalInput").ap()
    # [mag | ones] value matrix, chunk-major [nvch, 128, vchunk*65]
    mo = nc.dram_tensor("mo", [nvch, 128, vchunk * 65], F32R,
                        kind="ExternalInput").ap()
    # phase value matrix, chunk-major [nvch, 128, vchunk*64]
    pv = nc.dram_tensor("pv", [nvch, 128, vchunk * d], F32R,
                        kind="ExternalInput").ap()

    om = nc.dram_tensor("om", [65, q], F32, kind="ExternalOutput").ap()
    op = nc.dram_tensor("op", [d, q], F32, kind="ExternalOutput").ap()

    with tile.TileContext(nc) as tc, ExitStack() as ctx:
        const = ctx.enter_context(tc.tile_pool(name="const", bufs=1))
        persist = ctx.enter_context(tc.tile_pool(name="persist", bufs=1))
        bpool = ctx.enter_context(tc.tile_pool(name="build", bufs=3))
        epool = ctx.enter_context(tc.tile_pool(name="exps", bufs=6))
        spool = ctx.enter_context(tc.tile_pool(name="scores", bufs=2, space="PSUM"))
        apool = ctx.enter_context(tc.tile_pool(name="accum", bufs=1, space="PSUM"))

        abt = persist.tile([128, n], F32R)       # [a|b]^T for all keys
        mo_t = persist.tile([128, kblocks, 65], F32R)
        pv_t = persist.tile([128, kblocks, d], F32R)

        # ---- build ab^T: ab[0:64] = mag*cos(phase), ab[64:128] = mag*sin(phase)
        # phase args ride the sync HWDGE queue, mags the gpsimd SWDGE queue.
        for ci in range(nbch):
            c0 = ci * bchunk
            sl = slice(c0, c0 + bchunk)
            ph = bpool.tile([128, bchunk], F16, tag="ph")
            nc.sync.dma_start(out=ph[:, :], in_=ph2[ci, :, :])
            mg = bpool.tile([128, bchunk], F16, tag="mg")
            nc.gpsimd.dma_start(out=mg[:, :], in_=mg2[ci, :, :])
            tr = bpool.tile([128, bchunk], F32, tag="tr")
            nc.scalar.activation(
                tr[:, :], ph[:, :], mybir.ActivationFunctionType.Sin,
            )
            nc.vector.tensor_mul(abt[:, sl], mg[:, :], tr[:, :])

        abq = abt[:, 0:q]          # queries are the first q key columns

        # value matrices: mo behind the phases on sync, pv behind the mags
        # on gpsimd, chunked so early key blocks unblock the first value
        # matmuls quickly
        # value matrices on gpsimd BEHIND the mags -- the sync queue must
        # carry nothing but the phase chunks, which gate the ACT sin stream
        for vi in range(nvch):
            b0 = vi * vchunk
            b1 = b0 + vchunk
            nc.gpsimd.dma_start(out=mo_t[:, b0:b1, :], in_=mo[vi, :, :])
            nc.gpsimd.dma_start(out=pv_t[:, b0:b1, :], in_=pv[vi, :, :])

        # ---- main loop: all q (both 512-halves) in flight at once.
        # Per key block kb: one stationary load of abt_k shared by the two
        # score matmuls (q halves), ONE wide [128, q] exp, value matmuls two
        # key blocks behind (their es-ready semaphores are then already
        # satisfied when the weight loads issue).
        # PSUM budget: scores [128,1024]x2bufs = 4 banks, psA/psB = 4 banks.
        assert q == 2 * qblk
        # one accumulator tile (= one PSUM bank / zero-region) per q half
        psA = [apool.tile([65, qblk], F32, name=f"psA{j}", tag=f"psA{j}")
               for j in range(2)]
        psB = [apool.tile([64, qblk], F32, name=f"psB{j}", tag=f"psB{j}")
               for j in range(2)]

        def value_mms(es, kb, first, last):
            for m_t, ps in ((mo_t, psA), (pv_t, psB)):
                for j in range(2):
                    nc.tensor.matmul(
                        out=ps[j][:, :],
                        lhsT=m_t[:, kb, :],
                        rhs=es[:, j * qblk:(j + 1) * qblk],
                        start=first, stop=last,
                    )

        # zero bias tile whose producer reads the tail column of every abt
        # chunk -> transitively depends on every Sin. This keeps all Sins
        # scheduled before the first Exp on ACT: Sin and Exp live in
        # different ACT table sets, so interleaving costs a ~2.7us table
        # reload per switch.
        abt_tails = abt[:, bchunk - 1::bchunk]    # [128, nbch] strided
        zb = const.tile([128, 1], F32)
        ztmp = const.tile([128, nbch], F32)
        nc.vector.tensor_scalar_mul(ztmp[:, :], abt_tails[:, :], 0.0)
        nc.vector.tensor_reduce(
            out=zb[:, :], in_=ztmp[:, :], axis=mybir.AxisListType.X,
            op=mybir.AluOpType.add)

        es_hist = []
        for kb in range(kblocks):
            ss = spool.tile([128, q], F32)
            for j in range(2):
                nc.tensor.matmul(
                    out=ss[:, j * qblk:(j + 1) * qblk],
                    lhsT=abt[:, kb * 128:(kb + 1) * 128],
                    rhs=abq[:, j * qblk:(j + 1) * qblk],
                    start=True, stop=True,
                )
            es = epool.tile([128, q], F32R)
            nc.scalar.activation(
                es[:, :], ss[:, :], mybir.ActivationFunctionType.Exp,
                bias=zb[:, :],
            )
            if len(es_hist) >= 2:
                value_mms(es_hist[-2], kb - 2, first=(kb == 2), last=False)
            es_hist.append(es)
        value_mms(es_hist[-2], kblocks - 2, first=False, last=False)
        value_mms(es_hist[-1], kblocks - 1, first=False, last=True)

        # outputs: PSUM -> SBUF (DVE) -> DRAM
        opool = ctx.enter_context(tc.tile_pool(name="outs", bufs=2))
        for j in range(2):
            qsl = slice(j * qblk, (j + 1) * qblk)
            oA = opool.tile([65, qblk], F32, tag="oA")
            nc.vector.tensor_copy(oA[:, :], psA[j][:, :])
            nc.sync.dma_start(out=om[:, qsl], in_=oA[:, :])
            oB = opool.tile([64, qblk], F32, tag="oB")
            nc.vector.tensor_copy(oB[:, :], psB[j][:, :])
            nc.sync.dma_start(out=op[:, qsl], in_=oB[:, :])

    nc.compile()
    return nc


def make_inputs(mag, phase, n_cores=8):
    """Host-side sharding/layout prep -> per-core (key-rotated) input maps."""
    n, d = mag.shape
    q = n // n_cores
    kblocks = n // 128
    mag = np.ascontiguousarray(mag, dtype=np.float32)
    phase = np.ascontiguousarray(phase, dtype=np.float32)

    def wrap(x):
        # into [-pi, pi) -- ScalarE Sin domain
        return ((x + np.pi) % (2.0 * np.pi) - np.pi).astype(np.float32)

    ph2_g = np.concatenate(
        [wrap(phase.T + HALF_PI), wrap(phase.T)], axis=0).astype(np.float16)
    mg2_g = np.concatenate([mag.T, mag.T], axis=0).astype(np.float16)
    mo_nat = np.concatenate([mag, np.ones((n, 1), np.float32)], axis=1)

    bchunk = min(2048, n)
    nbch = n // bchunk
    vchunk = max(1, kblocks // 8)
    nvch = kblocks // vchunk

    def chunk_tr(x):  # [128, n] -> [nbch, 128, bchunk] chunk-major
        return np.ascontiguousarray(
            x.reshape(2 * d, nbch, bchunk).transpose(1, 0, 2))

    def tile_nat(x):  # [n, m] -> [nvch, 128, vchunk*m] chunk-major
        m = x.shape[1]
        y = x.reshape(nvch, vchunk, 128, m).transpose(0, 2, 1, 3)
        return np.ascontiguousarray(y.reshape(nvch, 128, vchunk * m))

    in_maps = []
    for c in range(n_cores):
        r = c * q
        in_maps.append({
            "ph2": chunk_tr(np.roll(ph2_g, -r, axis=1)),
            "mg2": chunk_tr(np.roll(mg2_g, -r, axis=1)),
            "mo": tile_nat(np.roll(mo_nat, -r, axis=0)),
            "pv": tile_nat(np.roll(phase, -r, axis=0)),
        })
    return in_maps


def gather_outputs(results, n, d, n_cores=8):
    """Per-core [65,q]/[64,q] transposed unnormalized sums -> full outputs."""
    new_mag = np.empty((n, d), np.float32)
    new_phase = np.empty((n, d), np.float32)
    q = n // n_cores
    for c in range(n_cores):
        om = results[c]["om"]          # [65, q]
        op = results[c]["op"]          # [64, q]
        den = om[64:65, :]             # [1, q]
        qsl = slice(c * q, (c + 1) * q)
        new_mag[qsl] = (om[:64, :] / den).T
        new_phase[qsl] = (op / den).T
    return new_mag, new_phase


_PROGRAM_CACHE = {}


def _get_program(n, d, n_cores):
    key = (n, d, n_cores)
    if key not in _PROGRAM_CACHE:
        _PROGRAM_CACHE[key] = build_program(n=n, d=d, n_cores=n_cores)
    return _PROGRAM_CACHE[key]


def kernel(mag, phase):
    mag = np.asarray(mag, dtype=np.float32)
    phase = np.asarray(phase, dtype=np.float32)
    n, d = mag.shape
    n_cores = 8
    nc = _get_program(n, d, n_cores)
    in_maps = make_inputs(mag, phase, n_cores=n_cores)
    res = run_bass_kernel_spmd(nc, in_maps, list(range(n_cores)))
    return gather_outputs(res.results, n, d, n_cores=n_cores)


# revision 26
# speedup vs baseline: 1.2802x; 1.0168x over previous
"""ComplexPolarAttention Trainium2 kernel.

score_ij = sum_d mag_i,d mag_j,d cos(phase_i,d - phase_j,d)
         = a_i . a_j + b_i . b_j          with a = mag*cos(phase), b = mag*sin(phase)
out_mag   = softmax(score, axis=1) @ mag
out_phase = softmax(score, axis=1) @ phase

Strategy (8 NeuronCores, SPMD, no collectives):
  - Rows (queries) sharded; keys replicated. Per-core inputs are ROTATED
    along the key axis so that core c's queries are always columns 0..q of
    its own key panel (softmax over keys is permutation invariant), so the
    query operand is just a slice of the key panel.
  - Every core builds the packed ab^T = [a|b]^T [128=2D, N] on-chip (one
    ACT Sin over host-prewrapped fp16 phase args, one DVE mul with fp16
    mags), fusing the two score GEMMs into ONE K=128 fp32r matmul per key
    block of 128.
  - Scores are computed transposed, S^T[k_blk=128, q] in PSUM (one wide
    [128, 1024] exp per key block amortizes ACT's ~352-cycle instruction
    overhead; scores are bounded by D=64 < 88 so unnormalized exp cannot
    overflow), then used as the MOVING operand of the value matmuls whose
    stationary operands are natural-layout [mag|ones] (the ones column
    yields the softmax denominator for free) and phase.
  - PSUM accumulates the numerators over all 64 key blocks; the final
    divide by the denominator happens on host during the gather.
"""

import numpy as np
from contextlib import ExitStack

import concourse.bass as bass
import concourse.tile as tile
from concourse import bacc, mybir
from concourse.bass_utils import run_bass_kernel_spmd

F32 = mybir.dt.float32
F32R = mybir.dt.float32r
F16 = mybir.dt.float16
HALF_PI = float(np.pi / 2.0)


def build_program(n=8192, d=64, n_cores=8, enable_asserts=False):
    """Build the SPMD Bass program. Every core runs identical IR; per-core
    behavior comes only from per-core (rotated) input data."""
    assert d == 64
    q = n // n_cores            # queries per core
    kblocks = n // 128          # key blocks of 128
    qblk = q // 2               # half processed per matmul (fp32 moving max 512)
    assert qblk <= 512 and n % 128 == 0

    nc = bacc.Bacc(
        "TRN2",
        target_bir_lowering=False,
        debug=False,
        enable_asserts=enable_asserts,
        num_devices=n_cores,
    )

    # ---- DRAM I/O (all per-core arrays rotated so queries = keys[0:q]) ----
    # ph2: [wrap(phase^T + pi/2) | wrap(phase^T)] stacked to 128 partitions,
    # wrapped into [-pi, pi) on host (ScalarE Sin domain), fp16.
    # mg2: [mag^T | mag^T] stacked, fp16.
    bchunk = min(2048, n)
    nbch = n // bchunk
    vchunk = max(1, kblocks // 8)
    nvch = kblocks // vchunk
    # all inputs chunk-major so every dma_start reads one fully contiguous
    # DRAM block (strided per-partition slices are descriptor-dominated)
    ph2 = nc.dram_tensor("ph2", [nbch, 2 * d, bchunk], F16,
                         kind="ExternalInput").ap()
    mg2 = nc.dram_tensor("mg2", [nbch, 2 * d, bchunk], F16,
                         kind="ExternalInput").ap()
    # [mag | ones] value matrix, chunk-major [nvch, 128, vchunk*65]
    mo = nc.dram_tensor("mo", [nvch, 128, vchunk * 65], F32R,
                        kind="ExternalInput").ap()
    # phase value matrix, chunk-major [nvch, 128, vchunk*64]
    pv = nc.dram_tensor("pv", [nvch, 128, vchunk * d], F32R,
                        kind="ExternalInput").ap()

    om = nc.dram_tensor("om", [65, q], F32, kind="ExternalOutput").ap()
    op = nc.dram_tensor("op", [d, q], F32, kind="ExternalOutput").ap()

    with tile.TileContext(nc) as tc, ExitStack() as ctx:
        const = ctx.enter_context(tc.tile_pool(name="const", bufs=1))
        persist = ctx.enter_context(tc.tile_pool(name="persist", bufs=1))
        bpool = ctx.enter_context(tc.tile_pool(name="build", bufs=4))
        epool = ctx.enter_context(tc.tile_pool(name="exps", bufs=6))
        spool = ctx.enter_context(tc.tile_pool(name="scores", bufs=2, space="PSUM"))
        apool = ctx.enter_context(tc.tile_pool(name="accum", bufs=1, space="PSUM"))

        abt = persist.tile([128, n], F32R)       # [a|b]^T for all keys
        mo_t = persist.tile([128, kblocks, 65], F32R)
        pv_t = persist.tile([128, kblocks, d], F32R)

        # ---- build ab^T: ab[0:64] = mag*cos(phase), ab[64:128] = mag*sin(phase)
        # phase args ride the sync HWDGE queue, mags the gpsimd SWDGE queue.
        for ci in range(nbch):
            c0 = ci * bchunk
            sl = slice(c0, c0 + bchunk)
            ph = bpool.tile([128, bchunk], F16, tag="ph")
            nc.sync.dma_start(out=ph[:, :], in_=ph2[ci, :, :])
            mg = bpool.tile([128, bchunk], F16, tag="mg")
            nc.gpsimd.dma_start(out=mg[:, :], in_=mg2[ci, :, :])
            tr = bpool.tile([128, bchunk], F32, tag="tr")
            nc.scalar.activation(
                tr[:, :], ph[:, :], mybir.ActivationFunctionType.Sin,
            )
            nc.vector.tensor_mul(abt[:, sl], mg[:, :], tr[:, :])

        abq = abt[:, 0:q]          # queries are the first q key columns

        # value matrices: mo behind the phases on sync, pv behind the mags
        # on gpsimd, chunked so early key blocks unblock the first value
        # matmuls quickly
        # value matrices on gpsimd BEHIND the mags -- the sync queue must
        # carry nothing but the phase chunks, which gate the ACT sin stream
        for vi in range(nvch):
            b0 = vi * vchunk
            b1 = b0 + vchunk
            nc.gpsimd.dma_start(out=mo_t[:, b0:b1, :], in_=mo[vi, :, :])
            nc.gpsimd.dma_start(out=pv_t[:, b0:b1, :], in_=pv[vi, :, :])

        # ---- main loop: all q (both 512-halves) in flight at once.
        # Per key block kb: one stationary load of abt_k shared by the two
        # score matmuls (q halves), ONE wide [128, q] exp, value matmuls two
        # key blocks behind (their es-ready semaphores are then already
        # satisfied when the weight loads issue).
        # PSUM budget: scores [128,1024]x2bufs = 4 banks, psA/psB = 4 banks.
        assert q == 2 * qblk
        # one accumulator tile (= one PSUM bank / zero-region) per q half
        psA = [apool.tile([65, qblk], F32, name=f"psA{j}", tag=f"psA{j}")
               for j in range(2)]
        psB = [apool.tile([64, qblk], F32, name=f"psB{j}", tag=f"psB{j}")
               for j in range(2)]

        def value_mms(es, kb, first, last):
            for m_t, ps in ((mo_t, psA), (pv_t, psB)):
                for j in range(2):
                    nc.tensor.matmul(
                        out=ps[j][:, :],
                        lhsT=m_t[:, kb, :],
                        rhs=es[:, j * qblk:(j + 1) * qblk],
                        start=first, stop=last,
                    )

        # zero bias tile whose producer reads the tail column of every abt
        # chunk -> transitively depends on every Sin. This keeps all Sins
        # scheduled before the first Exp on ACT: Sin and Exp live in
        # different ACT table sets, so interleaving costs a ~2.7us table
        # reload per switch.
        abt_tails = abt[:, bchunk - 1::bchunk]    # [128, nbch] strided
        zb = const.tile([128, 1], F32)
        ztmp = const.tile([128, nbch], F32)
        nc.vector.tensor_scalar_mul(ztmp[:, :], abt_tails[:, :], 0.0)
        nc.vector.tensor_reduce(
            out=zb[:, :], in_=ztmp[:, :], axis=mybir.AxisListType.X,
            op=mybir.AluOpType.add)

        es_hist = []
        for kb in range(kblocks):
            ss = spool.tile([128, q], F32)
            for j in range(2):
                nc.tensor.matmul(
                    out=ss[:, j * qblk:(j + 1) * qblk],
                    lhsT=abt[:, kb * 128:(kb + 1) * 128],
                    rhs=abq[:, j * qblk:(j + 1) * qblk],
                    start=True, stop=True,
                )
            es = epool.tile([128, q], F32R)
            nc.scalar.activation(
                es[:, :], ss[:, :], mybir.ActivationFunctionType.Exp,
                bias=zb[:, :],
            )
            if len(es_hist) >= 2:
                value_mms(es_hist[-2], kb - 2, first=(kb == 2), last=False)
            es_hist.append(es)
        value_mms(es_hist[-2], kblocks - 2, first=False, last=False)
        value_mms(es_hist[-1], kblocks - 1, first=False, last=True)

        # outputs: PSUM -> SBUF (DVE) -> DRAM
        opool = ctx.enter_context(tc.tile_pool(name="outs", bufs=2))
        for j in range(2):
            qsl = slice(j * qblk, (j + 1) * qblk)
            oA = opool.tile([65, qblk], F32, tag="oA")
            nc.vector.tensor_copy(oA[:, :], psA[j][:, :])
            nc.sync.dma_start(out=om[:, qsl], in_=oA[:, :])
            oB = opool.tile([64, qblk], F32, tag="oB")
            nc.vector.tensor_copy(oB[:, :], psB[j][:, :])
            nc.sync.dma_start(out=op[:, qsl], in_=oB[:, :])

    nc.compile()
    return nc


def make_inputs(mag, phase, n_cores=8):
    """Host-side sharding/layout prep -> per-core (key-rotated) input maps."""
    n, d = mag.shape
    q = n // n_cores
    kblocks = n // 128
    mag = np.ascontiguousarray(mag, dtype=np.float32)
    phase = np.ascontiguousarray(phase, dtype=np.float32)

    def wrap(x):
        # into [-pi, pi) -- ScalarE Sin domain
        return ((x + np.pi) % (2.0 * np.pi) - np.pi).astype(np.float32)

    ph2_g = np.concatenate(
        [wrap(phase.T + HALF_PI), wrap(phase.T)], axis=0).astype(np.float16)
    mg2_g = np.concatenate([mag.T, mag.T], axis=0).astype(np.float16)
    mo_nat = np.concatenate([mag, np.ones((n, 1), np.float32)], axis=1)

    bchunk = min(2048, n)
    nbch = n // bchunk
    vchunk = max(1, kblocks // 8)
    nvch = kblocks // vchunk

    def chunk_tr(x):  # [128, n] -> [nbch, 128, bchunk] chunk-major
        return np.ascontiguousarray(
            x.reshape(2 * d, nbch, bchunk).transpose(1, 0, 2))

    def tile_nat(x):  # [n, m] -> [nvch, 128, vchunk*m] chunk-major
        m = x.shape[1]
        y = x.reshape(nvch, vchunk, 128, m).transpose(0, 2, 1, 3)
        return np.ascontiguousarray(y.reshape(nvch, 128, vchunk * m))

    in_maps = []
    for c in range(n_cores):
        r = c * q
        in_maps.append({
            "ph2": chunk_tr(np.roll(ph2_g, -r, axis=1)),
            "mg2": chunk_tr(np.roll(mg2_g, -r, axis=1)),
            "mo": tile_nat(np.roll(mo_nat, -r, axis=0)),
            "pv": tile_nat(np.roll(phase, -r, axis=0)),
        })
    return in_maps


def gather_outputs(results, n, d, n_cores=8):
    """Per-core [65,q]/[64,q] transposed unnormalized sums -> full outputs."""
    new_mag = np.empty((n, d), np.float32)
    new_phase = np.empty((n, d), np.float32)
    q = n // n_cores
    for c in range(n_cores):
        om = results[c]["om"]          # [65, q]
        op = results[c]["op"]          # [64, q]
        den = om[64:65, :]             # [1, q]
        qsl = slice(c * q, (c + 1) * q)
        new_mag[qsl] = (om[:64, :] / den).T
        new_phase[qsl] = (op / den).T
    return new_mag, new_phase


_PROGRAM_CACHE = {}


def _get_program(n, d, n_cores):
    key = (n, d, n_cores)
    if key not in _PROGRAM_CACHE:
        _PROGRAM_CACHE[key] = build_program(n=n, d=d, n_cores=n_cores)
    return _PROGRAM_CACHE[key]


def kernel(mag, phase):
    mag = np.asarray(mag, dtype=np.float32)
    phase = np.asarray(phase, dtype=np.float32)
    n, d = mag.shape
    n_cores = 8
    nc = _get_program(n, d, n_cores)
    in_maps = make_inputs(mag, phase, n_cores=n_cores)
    res = run_bass_kernel_spmd(nc, in_maps, list(range(n_cores)))
    return gather_outputs(res.results, n, d, n_cores=n_cores)


# revision 27
# speedup vs baseline: 1.4502x; 1.1328x over previous
"""ComplexPolarAttention Trainium2 kernel.

score_ij = sum_d mag_i,d mag_j,d cos(phase_i,d - phase_j,d)
         = a_i . a_j + b_i . b_j          with a = mag*cos(phase), b = mag*sin(phase)
out_mag   = softmax(score, axis=1) @ mag
out_phase = softmax(score, axis=1) @ phase

Strategy (8 NeuronCores, SPMD, no collectives):
  - Rows (queries) sharded; keys replicated. Per-core inputs are ROTATED
    along the key axis so that core c's queries are always columns 0..q of
    its own key panel (softmax over keys is permutation invariant), so the
    query operand is just a slice of the key panel.
  - The packed ab^T = [a|b]^T [128=2D, N] panel (host-prepped layout) fuses
    the two score GEMMs into ONE K=128 fp32r matmul per key block of 128.
  - Scores are computed transposed, S^T[k_blk=128, q] in PSUM (one wide
    [128, 1024] exp per key block amortizes ACT's ~352-cycle instruction
    overhead; scores are bounded by D=64 < 88 so unnormalized exp cannot
    overflow), then used as the MOVING operand of the value matmuls whose
    stationary operands are natural-layout [mag|ones] (the ones column
    yields the softmax denominator for free) and phase.
  - PSUM accumulates the numerators over all 64 key blocks; the final
    divide by the denominator happens on host during the gather.
  - All DRAM inputs are chunk-major so every dma_start reads one fully
    contiguous block; the ab^T chunks ride the sync HWDGE queue, the value
    matrices the gpsimd SWDGE queue, so the k-loop's critical first chunk
    lands as early as possible and later chunks stream in under compute.
"""

import numpy as np
from contextlib import ExitStack

import concourse.bass as bass
import concourse.tile as tile
from concourse import bacc, mybir
from concourse.bass_utils import run_bass_kernel_spmd

F32 = mybir.dt.float32
F32R = mybir.dt.float32r


def build_program(n=8192, d=64, n_cores=8, enable_asserts=False):
    """Build the SPMD Bass program. Every core runs identical IR; per-core
    behavior comes only from per-core (rotated) input data."""
    assert d == 64
    q = n // n_cores            # queries per core
    kblocks = n // 128          # key blocks of 128
    qblk = q // 2               # half processed per matmul (fp32 moving max 512)
    assert qblk <= 512 and n % 128 == 0

    nc = bacc.Bacc(
        "TRN2",
        target_bir_lowering=False,
        debug=False,
        enable_asserts=enable_asserts,
        num_devices=n_cores,
    )

    # ---- DRAM I/O (all per-core arrays rotated so queries = keys[0:q]) ----
    abchunk = min(1024, n)
    nabch = n // abchunk
    vchunk = max(1, kblocks // 16)
    nvch = kblocks // vchunk
    # packed [a|b]^T panel, chunk-major [nabch, 128, abchunk]
    abt_in = nc.dram_tensor("abt", [nabch, 128, abchunk], F32R,
                            kind="ExternalInput").ap()
    # [mag | ones] value matrix, chunk-major [nvch, 128, vchunk*65]
    mo = nc.dram_tensor("mo", [nvch, 128, vchunk * 65], F32R,
                        kind="ExternalInput").ap()
    # phase value matrix, chunk-major [nvch, 128, vchunk*64]
    pv = nc.dram_tensor("pv", [nvch, 128, vchunk * d], F32R,
                        kind="ExternalInput").ap()

    om = nc.dram_tensor("om", [65, q], F32, kind="ExternalOutput").ap()
    op = nc.dram_tensor("op", [d, q], F32, kind="ExternalOutput").ap()

    with tile.TileContext(nc) as tc, ExitStack() as ctx:
        persist = ctx.enter_context(tc.tile_pool(name="persist", bufs=1))
        epool = ctx.enter_context(tc.tile_pool(name="exps", bufs=6))
        opool = ctx.enter_context(tc.tile_pool(name="outs", bufs=2))
        spool = ctx.enter_context(tc.tile_pool(name="scores", bufs=2, space="PSUM"))
        apool = ctx.enter_context(tc.tile_pool(name="accum", bufs=1, space="PSUM"))

        abt = persist.tile([128, n], F32R)       # [a|b]^T for all keys
        mo_t = persist.tile([128, kblocks, 65], F32R)
        pv_t = persist.tile([128, kblocks, d], F32R)

        # ab^T chunks on the sync queue -- chunk 0 (== the query slice)
        # gates the first score matmul and exp
        for ci in range(nabch):
            nc.sync.dma_start(
                out=abt[:, ci * abchunk:(ci + 1) * abchunk],
                in_=abt_in[ci, :, :])
        abq = abt[:, 0:q]          # queries are the first q key columns

        # value matrices on the gpsimd queue, fine-grained and interleaved
        # so the first key blocks' stationaries land just after exp0
        for vi in range(nvch):
            b0 = vi * vchunk
            b1 = b0 + vchunk
            nc.gpsimd.dma_start(out=mo_t[:, b0:b1, :], in_=mo[vi, :, :])
            nc.gpsimd.dma_start(out=pv_t[:, b0:b1, :], in_=pv[vi, :, :])

        # ---- main loop: all q (both 512-halves) in flight at once.
        # Per key block kb: one stationary load of abt_k shared by the two
        # score matmuls (q halves), ONE wide [128, q] exp, value matmuls two
        # key blocks behind (their es-ready semaphores are then already
        # satisfied when the weight loads issue).
        # PSUM budget: scores [128,1024]x2bufs = 4 banks, psA/psB = 4 banks.
        assert q == 2 * qblk
        # one accumulator tile (= one PSUM bank / zero-region) per q half
        psA = [apool.tile([65, qblk], F32, name=f"psA{j}", tag=f"psA{j}")
               for j in range(2)]
        psB = [apool.tile([64, qblk], F32, name=f"psB{j}", tag=f"psB{j}")
               for j in range(2)]

        def value_mms(es, kb, first, last):
            for m_t, ps in ((mo_t, psA), (pv_t, psB)):
                for j in range(2):
                    nc.tensor.matmul(
                        out=ps[j][:, :],
                        lhsT=m_t[:, kb, :],
                        rhs=es[:, j * qblk:(j + 1) * qblk],
                        start=first, stop=last,
                    )

        es_hist = []
        for kb in range(kblocks):
            ss = spool.tile([128, q], F32)
            for j in range(2):
                nc.tensor.matmul(
                    out=ss[:, j * qblk:(j + 1) * qblk],
                    lhsT=abt[:, kb * 128:(kb + 1) * 128],
                    rhs=abq[:, j * qblk:(j + 1) * qblk],
                    start=True, stop=True,
                )
            es = epool.tile([128, q], F32R)
            nc.scalar.activation(
                es[:, :], ss[:, :], mybir.ActivationFunctionType.Exp,
            )
            if len(es_hist) >= 2:
                value_mms(es_hist[-2], kb - 2, first=(kb == 2), last=False)
            es_hist.append(es)
        value_mms(es_hist[-2], kblocks - 2, first=False, last=False)
        value_mms(es_hist[-1], kblocks - 1, first=False, last=True)

        # outputs: PSUM -> SBUF (DVE) -> DRAM
        for j in range(2):
            qsl = slice(j * qblk, (j + 1) * qblk)
            oA = opool.tile([65, qblk], F32, tag="oA")
            nc.vector.tensor_copy(oA[:, :], psA[j][:, :])
            nc.sync.dma_start(out=om[:, qsl], in_=oA[:, :])
            oB = opool.tile([64, qblk], F32, tag="oB")
            nc.vector.tensor_copy(oB[:, :], psB[j][:, :])
            nc.sync.dma_start(out=op[:, qsl], in_=oB[:, :])

    nc.compile()
    return nc


def make_inputs(mag, phase, n_cores=8):
    """Host-side sharding/layout prep -> per-core (key-rotated) input maps."""
    n, d = mag.shape
    q = n // n_cores
    kblocks = n // 128
    mag = np.ascontiguousarray(mag, dtype=np.float32)
    phase = np.ascontiguousarray(phase, dtype=np.float32)

    a = mag * np.cos(phase)
    b = mag * np.sin(phase)
    abt_g = np.concatenate([a.T, b.T], axis=0).astype(np.float32)  # [128, n]
    mo_nat = np.concatenate([mag, np.ones((n, 1), np.float32)], axis=1)

    abchunk = min(1024, n)
    nabch = n // abchunk
    vchunk = max(1, kblocks // 16)
    nvch = kblocks // vchunk

    def chunk_tr(x):  # [128, n] -> [nabch, 128, abchunk] chunk-major
        return np.ascontiguousarray(
            x.reshape(128, nabch, abchunk).transpose(1, 0, 2))

    def tile_nat(x):  # [n, m] -> [nvch, 128, vchunk*m] chunk-major
        m = x.shape[1]
        y = x.reshape(nvch, vchunk, 128, m).transpose(0, 2, 1, 3)
        return np.ascontiguousarray(y.reshape(nvch, 128, vchunk * m))

    in_maps = []
    for c in range(n_cores):
        r = c * q
        in_maps.append({
            "abt": chunk_tr(np.roll(abt_g, -r, axis=1)),
            "mo": tile_nat(np.roll(mo_nat, -r, axis=0)),
            "pv": tile_nat(np.roll(phase, -r, axis=0)),
        })
    return in_maps


def gather_outputs(results, n, d, n_cores=8):
    """Per-core [65,q]/[64,q] transposed unnormalized sums -> full outputs."""
    new_mag = np.empty((n, d), np.float32)
    new_phase = np.empty((n, d), np.float32)
    q = n // n_cores
    for c in range(n_cores):
        om = results[c]["om"]          # [65, q]
        op = results[c]["op"]          # [64, q]
        den = om[64:65, :]             # [1, q]
        qsl = slice(c * q, (c + 1) * q)
        new_mag[qsl] = (om[:64, :] / den).T
        new_phase[qsl] = (op / den).T
    return new_mag, new_phase


_PROGRAM_CACHE = {}


def _get_program(n, d, n_cores):
    key = (n, d, n_cores)
    if key not in _PROGRAM_CACHE:
        _PROGRAM_CACHE[key] = build_program(n=n, d=d, n_cores=n_cores)
    return _PROGRAM_CACHE[key]


def kernel(mag, phase):
    mag = np.asarray(mag, dtype=np.float32)
    phase = np.asarray(phase, dtype=np.float32)
    n, d = mag.shape
    n_cores = 8
    nc = _get_program(n, d, n_cores)
    in_maps = make_inputs(mag, phase, n_cores=n_cores)
    res = run_bass_kernel_spmd(nc, in_maps, list(range(n_cores)))
    return gather_outputs(res.results, n, d, n_cores=n_cores)
